# revision 1
# baseline (speedup 1.0000x reference)
"""DeepSpeed-style self-attention block (RMSNorm + QKV + RoPE + causal attention
+ output projection) on 8 Trainium2 NeuronCores.

Sharding: tensor-parallel over heads (16 heads -> 2 per core). Each core computes
its 2 heads' attention over the full sequence and a partial output projection over
its 256-dim slice of the context; the 8 partial outputs are summed on the host
(the TP all-reduce equivalent, done at gather time).

Layouts (per core, all device matmuls in float32r: full PE rate, ~1e-4 rms rounding):
  xT      [2048, 4096]   x transposed (d-major) so projections need no on-device transpose
  wqT/wkT/wvT [2048, 256] weight slices pre-transposed on host, RMSNorm weight folded in
  woT     [256, 2048]    o-proj slice pre-transposed
  cosT/sinT [128, 4096]  RoPE tables (sin sign-folded); scaled on device by the
                         per-token RMS factor s_n so RoPE eviction applies the norm free
  masks   [4, 128, 512]  causal -10000 masks for the 4 diagonal k-tile offsets

Per 512-token chunk: one pass over 16 d-tiles accumulates qT/kT (per head) and
v (transposed layout) in PSUM plus the x^2 column sums ([1,512] via ones-matmul);
s = rsqrt(mean+eps) feeds the table scaling, the v eviction (via PE transpose to
natural [token, dv] layout + tensor_scalar), and RoPE eviction of q/k. Attention
runs in scores-transposed layout [k,q] (softmax denominator = ones-matmul over the
exp tiles, accumulated alongside probs@v in PSUM), so no max-subtraction and no
transposes are needed (scores are bounded, exp(-10000 + s) underflows to exactly 0).
"""
import sys
sys.path.insert(0, '/opt/trn_rl_repo')

import math
import numpy as np
from contextlib import ExitStack

import concourse.bass as bass
from concourse import bacc
import concourse.mybir as mybir
import concourse.tile as tile
from concourse import bass_utils
from concourse.masks import make_identity

# ---- problem constants (hardcoded per contest contract) ----
B, S, H, HEADS, D = 2, 2048, 2048, 16, 128
NT = B * S                    # 4096 tokens
NCORES = 8
HPC = HEADS // NCORES         # 2 heads per core
OC = HPC * D                  # 256 output dims per core
P = 128
CH = 512                      # token chunk
NCH = NT // CH                # 8 chunks
KT = H // P                   # 16 d-tiles
CPB = S // CH                 # 4 chunks per batch
SCALE = 1.0 / math.sqrt(D)
RMS_EPS = 1e-6
ROPE_BASE = 10000.0
MASK_VAL = -10000.0

F32 = mybir.dt.float32
F32R = mybir.dt.float32r
EXP = mybir.ActivationFunctionType.Exp
SQRT = mybir.ActivationFunctionType.Sqrt


def build_module():
    nc = bacc.Bacc("TRN2", target_bir_lowering=False, debug=False, num_devices=NCORES)

    xT = nc.dram_tensor("xT", [H, NT], F32R, kind="ExternalInput").ap()
    wqT = nc.dram_tensor("wqT", [H, OC], F32R, kind="ExternalInput").ap()
    wkT = nc.dram_tensor("wkT", [H, OC], F32R, kind="ExternalInput").ap()
    wvT = nc.dram_tensor("wvT", [H, OC], F32R, kind="ExternalInput").ap()
    woT = nc.dram_tensor("woT", [OC, H], F32R, kind="ExternalInput").ap()
    cosT = nc.dram_tensor("cosT", [D, NT], F32, kind="ExternalInput").ap()
    sinT = nc.dram_tensor("sinT", [D, NT], F32, kind="ExternalInput").ap()
    masks = nc.dram_tensor("masks", [CPB, P, CH], mybir.dt.bfloat16, kind="ExternalInput").ap()
    ones_col = nc.dram_tensor("ones_col", [P, 1], F32R, kind="ExternalInput").ap()
    out_p = nc.dram_tensor("out_p", [NT, H], F32, kind="ExternalOutput").ap()

    with tile.TileContext(nc) as tc, ExitStack() as ctx:
        const = ctx.enter_context(tc.tile_pool(name="const", bufs=1))
        wpool = ctx.enter_context(tc.tile_pool(name="wpool", bufs=1))
        kvpool = ctx.enter_context(tc.tile_pool(name="kvpool", bufs=1))
        xt_pool = ctx.enter_context(tc.tile_pool(name="xtp", bufs=4))
        sq_pool = ctx.enter_context(tc.tile_pool(name="sqp", bufs=2))
        trig = ctx.enter_context(tc.tile_pool(name="trig", bufs=2))
        rope_t = ctx.enter_context(tc.tile_pool(name="ropet", bufs=2))
        q_pool = ctx.enter_context(tc.tile_pool(name="qp", bufs=3))
        vt_pool = ctx.enter_context(tc.tile_pool(name="vtp", bufs=2))
        ex_pool = ctx.enter_context(tc.tile_pool(name="exp", bufs=3))
        ctx_pool = ctx.enter_context(tc.tile_pool(name="ctxp", bufs=2))
        o_pool = ctx.enter_context(tc.tile_pool(name="op", bufs=2))
        small1 = ctx.enter_context(tc.tile_pool(name="small1", bufs=1))
        small = ctx.enter_context(tc.tile_pool(name="small", bufs=2))
        bc_pool = ctx.enter_context(tc.tile_pool(name="bcp", bufs=2))
        ps = ctx.enter_context(tc.tile_pool(name="ps", bufs=8, space="PSUM"))

        # ---- resident constants ----
        ones_sb = const.tile([P, 1], F32R)
        nc.sync.dma_start(out=ones_sb, in_=ones_col)
        eps_sb = const.tile([1, 1], F32)
        nc.vector.memset(eps_sb, RMS_EPS)
        ident = const.tile([P, P], F32)
        make_identity(nc, ident)
        mask_sb = const.tile([P, CPB, CH], mybir.dt.bfloat16)
        nc.sync.dma_start(out=mask_sb, in_=masks.rearrange("j p q -> p j q"))
        wq_sb = wpool.tile([P, KT, OC], F32R)
        nc.sync.dma_start(out=wq_sb, in_=wqT.rearrange("(t p) o -> p t o", p=P))
        wk_sb = wpool.tile([P, KT, OC], F32R)
        nc.sync.dma_start(out=wk_sb, in_=wkT.rearrange("(t p) o -> p t o", p=P))
        wv_sb = wpool.tile([P, KT, OC], F32R)
        nc.sync.dma_start(out=wv_sb, in_=wvT.rearrange("(t p) o -> p t o", p=P))
        wo_sb = wpool.tile([P, HPC, H], F32R)
        nc.sync.dma_start(out=wo_sb, in_=woT.rearrange("(t p) o -> p t o", p=P))

        # per-chunk K/V caches, resident for the whole kernel
        k_chunks = [kvpool.tile([P, HPC, CH], F32R, name=f"kc{i}") for i in range(NCH)]
        v_chunks = [kvpool.tile([P, CPB, OC], F32R, name=f"vc{i}") for i in range(NCH)]

        pending = []

        def emit_oproj(m0, csb):
            for j in range(CPB):
                for oc in range(H // CH):
                    op_ = ps.tile([P, CH], F32, tag="ps", name=f"o{m0}_{j}_{oc}")
                    for h in range(HPC):
                        nc.tensor.matmul(op_, csb[:, h, j * P:(j + 1) * P],
                                         wo_sb[:, h, oc * CH:(oc + 1) * CH],
                                         start=(h == 0), stop=(h == HPC - 1))
                    ot = o_pool.tile([P, CH], F32, tag="ot")
                    nc.scalar.copy(ot, op_)
                    nc.sync.dma_start(
                        out=out_p[m0 + j * P:m0 + (j + 1) * P, oc * CH:(oc + 1) * CH],
                        in_=ot)

        for ich in range(NCH):
            b, li = ich // CPB, ich % CPB
            n0 = ich * CH

            # ---- pass A: projections + x^2 stats over 16 d-tiles ----
            qp = [ps.tile([P, CH], F32, tag="ps", name=f"qp{ich}_{h}") for h in range(HPC)]
            kp = [ps.tile([P, CH], F32, tag="ps", name=f"kp{ich}_{h}") for h in range(HPC)]
            vp = [ps.tile([P, CH], F32, tag="ps", name=f"vp{ich}_{h}") for h in range(HPC)]
            ssp = ps.tile([1, CH], F32, tag="ps", name=f"ss{ich}")
            for dt in range(KT):
                xt = xt_pool.tile([P, CH], F32R, tag="xt")
                nc.sync.dma_start(out=xt, in_=xT[dt * P:(dt + 1) * P, n0:n0 + CH])
                xq = sq_pool.tile([P, CH], F32R, tag="xq")
                nc.vector.tensor_mul(xq, xt, xt)
                st, sp = (dt == 0), (dt == KT - 1)
                for h in range(HPC):
                    nc.tensor.matmul(qp[h], wq_sb[:, dt, h * P:(h + 1) * P], xt, start=st, stop=sp)
                    nc.tensor.matmul(kp[h], wk_sb[:, dt, h * P:(h + 1) * P], xt, start=st, stop=sp)
                    nc.tensor.matmul(vp[h], wv_sb[:, dt, h * P:(h + 1) * P], xt, start=st, stop=sp)
                nc.tensor.matmul(ssp, ones_sb, xq, start=st, stop=sp)

            while pending:
                emit_oproj(*pending.pop(0))

            # ---- RMS scale: s = 1/sqrt(mean(x^2)+eps), row and column forms ----
            s_sqrt = small1.tile([1, CH], F32, tag="ssq")
            nc.scalar.activation(s_sqrt, ssp, SQRT, bias=eps_sb, scale=1.0 / H)
            s_row = small.tile([1, CH], F32, tag="srow")
            nc.vector.reciprocal(s_row, s_sqrt)
            s_bc = bc_pool.tile([P, CH], F32, tag="bc")
            nc.gpsimd.partition_broadcast(s_bc, s_row)

            cosS = trig.tile([P, CH], F32, tag="cosS")
            nc.sync.dma_start(out=cosS, in_=cosT[:, n0:n0 + CH])
            sinS = trig.tile([P, CH], F32, tag="sinS")
            nc.sync.dma_start(out=sinS, in_=sinT[:, n0:n0 + CH])
            nc.vector.tensor_mul(cosS, cosS, s_bc)
            nc.vector.tensor_mul(sinS, sinS, s_bc)

            # ---- RoPE + scale eviction of q, k (psum [d,512] -> f32r sbuf) ----
            HD = D // 2
            q_sb = []
            for h in range(HPC):
                for (psum_t, dst) in ((qp[h], None), (kp[h], k_chunks[ich][:, h, :])):
                    t1 = rope_t.tile([P, CH], F32, tag="t1")
                    nc.vector.tensor_mul(t1, psum_t, cosS)
                    t2 = rope_t.tile([P, CH], F32, tag="t2")
                    nc.vector.tensor_mul(t2[0:HD, :], psum_t[HD:P, :], sinS[0:HD, :])
                    nc.vector.tensor_mul(t2[HD:P, :], psum_t[0:HD, :], sinS[HD:P, :])
                    if dst is None:
                        dst = q_pool.tile([P, CH], F32R, tag="q")
                        q_sb.append(dst)
                    nc.vector.tensor_add(dst, t1, t2)

            # ---- V eviction: psum [o,512] -> transpose -> scaled natural [n, o] ----
            for h in range(HPC):
                vts = vt_pool.tile([P, CH], F32, tag="vts")
                nc.vector.tensor_mul(vts, vp[h], s_bc)
                for j in range(CPB):
                    tpp = ps.tile([P, P], F32, tag="ps", name=f"tp{ich}_{h}_{j}")
                    nc.tensor.transpose(tpp, vts[:, j * P:(j + 1) * P], ident)
                    nc.scalar.copy(v_chunks[ich][:, j, h * P:(h + 1) * P], tpp)

            # ---- attention for this q-chunk, per head ----
            nkt = CPB * (li + 1)
            ctx_sb = ctx_pool.tile([P, HPC, CH], F32R, tag="ctx")
            for h in range(HPC):
                ctxp = ps.tile([P, CH], F32, tag="ps", name=f"cx{ich}_{h}")
                denp = ps.tile([1, CH], F32, tag="ps", name=f"dn{ich}_{h}")
                for kt in range(nkt):
                    ck = b * CPB + kt // CPB
                    j = kt % CPB
                    sp_ = ps.tile([P, CH], F32, tag="ps", name=f"s{ich}_{h}_{kt}")
                    nc.tensor.matmul(sp_, k_chunks[ck][:, h, j * P:(j + 1) * P], q_sb[h],
                                     start=True, stop=True)
                    dj = kt - CPB * li
                    if dj >= 0:
                        nc.vector.tensor_add(sp_, sp_, mask_sb[:, dj, :])
                    ex = ex_pool.tile([P, CH], F32R, tag="ex")
                    nc.scalar.activation(ex, sp_, EXP, scale=SCALE)
                    st, last = (kt == 0), (kt == nkt - 1)
                    nc.tensor.matmul(ctxp, v_chunks[ck][:, j, h * P:(h + 1) * P], ex,
                                     start=st, stop=last)
                    nc.tensor.matmul(denp, ones_sb, ex, start=st, stop=last)
                den_s = small.tile([1, CH], F32, tag="dens")
                nc.scalar.copy(den_s, denp)
                rec = small.tile([1, CH], F32, tag="rec")
                nc.vector.reciprocal(rec, den_s)
                rbc = bc_pool.tile([P, CH], F32, tag="bc")
                nc.gpsimd.partition_broadcast(rbc, rec)
                nc.vector.tensor_mul(ctx_sb[:, h, :], ctxp, rbc)

            # ---- partial o-proj deferred into the next chunk's stats bubble ----
            pending.append((n0, ctx_sb))

        while pending:
            emit_oproj(*pending.pop(0))

    nc.compile()
    return nc


def prep_inputs(x, norm_w, wq, wk, wv, wo, position_ids):
    """Host-side sharding/layout prep. Returns per-core input maps."""
    x = np.asarray(x, dtype=np.float32)
    norm_w = np.asarray(norm_w, dtype=np.float32)
    wq = np.asarray(wq, dtype=np.float32)
    wk = np.asarray(wk, dtype=np.float32)
    wv = np.asarray(wv, dtype=np.float32)
    wo = np.asarray(wo, dtype=np.float32)
    pos = np.asarray(position_ids)

    xT = np.ascontiguousarray(x.reshape(NT, H).T)

    # RoPE tables from position_ids, sign-folded sin
    inv_freq = 1.0 / (ROPE_BASE ** (np.arange(0, D, 2, dtype=np.float32) / D))
    t = pos.reshape(NT).astype(np.float32)
    freqs = np.einsum("n,f->nf", t, inv_freq)
    emb = np.concatenate([freqs, freqs], axis=1)          # [NT, D]
    cos = np.cos(emb).astype(np.float32)
    sin = np.sin(emb).astype(np.float32)
    sinF = sin.copy()
    sinF[:, :D // 2] *= -1.0
    cosT = np.ascontiguousarray(cos.T)
    sinT = np.ascontiguousarray(sinF.T)

    # diagonal-block causal masks: mask[j][kk, qq] = 0 if qq >= j*128+kk else -1e4
    qq = np.arange(CH)[None, None, :]
    kk = np.arange(P)[None, :, None]
    jj = np.arange(CPB)[:, None, None]
    import ml_dtypes
    masks = np.where(qq >= jj * P + kk, 0.0, MASK_VAL).astype(ml_dtypes.bfloat16)

    ones_col = np.ones((P, 1), dtype=np.float32)

    wq_f = wq * norm_w[None, :]
    wk_f = wk * norm_w[None, :]
    wv_f = wv * norm_w[None, :]

    in_maps = []
    for c in range(NCORES):
        sl = slice(c * OC, (c + 1) * OC)
        in_maps.append({
            "xT": xT,
            "wqT": np.ascontiguousarray(wq_f[sl].T),
            "wkT": np.ascontiguousarray(wk_f[sl].T),
            "wvT": np.ascontiguousarray(wv_f[sl].T),
            "woT": np.ascontiguousarray(wo[:, sl].T),
            "cosT": cosT,
            "sinT": sinT,
            "masks": masks,
            "ones_col": ones_col,
        })
    return in_maps


_NC_CACHE = None


def _get_module():
    global _NC_CACHE
    if _NC_CACHE is None:
        _NC_CACHE = build_module()
    return _NC_CACHE


def kernel(x, norm_w, wq, wk, wv, wo, position_ids):
    nc = _get_module()
    in_maps = prep_inputs(x, norm_w, wq, wk, wv, wo, position_ids)
    res = bass_utils.run_bass_kernel_spmd(nc, in_maps, core_ids=list(range(NCORES)))
    acc = np.zeros((NT, H), dtype=np.float64)
    for c in range(NCORES):
        acc += res.results[c]["out_p"].astype(np.float64)
    return acc.astype(np.float32).reshape(B, S, H)



# revision 4
# speedup vs baseline: 1.3335x; 1.3335x over previous
"""DeepSpeed-style self-attention block (RMSNorm + QKV + RoPE + causal attention
+ output projection) on 8 Trainium2 NeuronCores.

Sharding: tensor-parallel over heads (16 heads -> 2 per core). Each core computes
its 2 heads' attention over the full sequence and a partial output projection over
its 256-dim slice of the context; the 8 partial outputs are summed on the host
(the TP all-reduce equivalent, done at gather time).

All PE matmuls run in bfloat16 (1 cyc/row at any ap size). Per-token RMS stats and
softmax denominators are computed with ap=1 matmuls (ones moving tensor, data as
stationary), which cost ~nothing on the PE. V is projected directly into natural
[token, dv] layout (x stationary, weights moving) so no PE transposes or extra
copies are needed; the RMS scale s is applied at V eviction via the activation
engine's per-partition scale operand. rsqrt is computed as exp(-0.5*ln(x)) so all
activation functions (Ln/Exp/Copy) live in one table set (no table reloads).
Scores/pv/exp are trimmed to the causal region at 128-column granularity, and the
causal mask shrinks to a single 128x128 triangle tile applied only on the exact
diagonal blocks. DMA is batched (3 loads + 1 store per 512-token chunk); output
stores issue from the activation engine's queue right after eviction.
"""
import sys
sys.path.insert(0, '/opt/trn_rl_repo')

import math
import numpy as np
from contextlib import ExitStack

import concourse.bass as bass
from concourse import bacc
import concourse.mybir as mybir
import concourse.tile as tile
from concourse import bass_utils
from concourse.masks import make_identity

# ---- problem constants (hardcoded per contest contract) ----
B, S, H, HEADS, D = 2, 2048, 2048, 16, 128
NT = B * S                    # 4096 tokens
NCORES = 8
HPC = HEADS // NCORES         # 2 heads per core
OC = HPC * D                  # 256 output dims per core
P = 128
CH = 512                      # token chunk
NCH = NT // CH                # 8 chunks
KT = H // P                   # 16 d-tiles
CPB = S // CH                 # 4 chunks per batch
HD = D // 2
SCALE = 1.0 / math.sqrt(D)
RMS_EPS = 1e-6
ROPE_BASE = 10000.0
MASK_VAL = -10000.0

F32 = mybir.dt.float32
BF16 = mybir.dt.bfloat16
EXP = mybir.ActivationFunctionType.Exp
LN = mybir.ActivationFunctionType.Ln
COPY = mybir.ActivationFunctionType.Copy


def build_module():
    nc = bacc.Bacc("TRN2", target_bir_lowering=False, debug=False, num_devices=NCORES)

    xT = nc.dram_tensor("xT", [H, NT], BF16, kind="ExternalInput").ap()
    wqT = nc.dram_tensor("wqT", [H, OC], BF16, kind="ExternalInput").ap()
    wkT = nc.dram_tensor("wkT", [H, OC], BF16, kind="ExternalInput").ap()
    wvT = nc.dram_tensor("wvT", [H, OC], BF16, kind="ExternalInput").ap()
    woT = nc.dram_tensor("woT", [OC, H], BF16, kind="ExternalInput").ap()
    trigT = nc.dram_tensor("trigT", [2, D, NT], BF16, kind="ExternalInput").ap()
    maskT = nc.dram_tensor("maskT", [P, P], BF16, kind="ExternalInput").ap()
    onesT = nc.dram_tensor("onesT", [P, 1], BF16, kind="ExternalInput").ap()
    out_p = nc.dram_tensor("out_p", [NT, H], BF16, kind="ExternalOutput").ap()

    xTr = xT.rearrange("(t p) n -> p t n", p=P)
    trig_r = trigT.rearrange("s d n -> d s n")
    out_pr = out_p.rearrange("(c j p) o -> c p j o", j=CPB, p=P)

    with tile.TileContext(nc) as tc, ExitStack() as ctx:
        const = ctx.enter_context(tc.tile_pool(name="const", bufs=1))
        wpool = ctx.enter_context(tc.tile_pool(name="wpool", bufs=1))
        kvpool = ctx.enter_context(tc.tile_pool(name="kvpool", bufs=1))
        xt_pool = ctx.enter_context(tc.tile_pool(name="xtp", bufs=2))
        xq_pool = ctx.enter_context(tc.tile_pool(name="xqp", bufs=3))
        trig_pool = ctx.enter_context(tc.tile_pool(name="trigp", bufs=2))
        trigs_pool = ctx.enter_context(tc.tile_pool(name="trigsp", bufs=2))
        sc_pool = ctx.enter_context(tc.tile_pool(name="scp", bufs=2))
        bc_pool = ctx.enter_context(tc.tile_pool(name="bcp", bufs=2))
        rope_t = ctx.enter_context(tc.tile_pool(name="ropet", bufs=2))
        q_pool = ctx.enter_context(tc.tile_pool(name="qp", bufs=3))
        ex_pool = ctx.enter_context(tc.tile_pool(name="exp", bufs=4))
        ctx_pool = ctx.enter_context(tc.tile_pool(name="ctxp", bufs=2))
        o_pool = ctx.enter_context(tc.tile_pool(name="op", bufs=2))
        ps = ctx.enter_context(tc.tile_pool(name="ps", bufs=8, space="PSUM"))

        # ---- resident constants ----
        ones_sb = const.tile([P, 1], BF16)
        nc.sync.dma_start(out=ones_sb, in_=onesT)
        ident = const.tile([P, P], F32)
        make_identity(nc, ident)
        eps_sb = const.tile([P, 1], F32)
        nc.vector.memset(eps_sb, RMS_EPS)
        mask_sb = const.tile([P, P], BF16)
        nc.sync.dma_start(out=mask_sb, in_=maskT)
        wq_sb = wpool.tile([P, KT, OC], BF16)
        nc.sync.dma_start(out=wq_sb, in_=wqT.rearrange("(t p) o -> p t o", p=P))
        wk_sb = wpool.tile([P, KT, OC], BF16)
        nc.sync.dma_start(out=wk_sb, in_=wkT.rearrange("(t p) o -> p t o", p=P))
        wv_sb = wpool.tile([P, KT, OC], BF16)
        nc.sync.dma_start(out=wv_sb, in_=wvT.rearrange("(t p) o -> p t o", p=P))
        wo_sb = wpool.tile([P, HPC, H], BF16)
        nc.sync.dma_start(out=wo_sb, in_=woT.rearrange("(t p) o -> p t o", p=P))

        # per-chunk K/V caches, resident for the whole kernel
        k_chunks = [kvpool.tile([P, HPC, CH], BF16, name=f"kc{i}") for i in range(NCH)]
        v_chunks = [kvpool.tile([P, CPB, OC], BF16, name=f"vc{i}") for i in range(NCH)]

        pending = []

        def emit_oproj(ich0, csb):
            o_sb = o_pool.tile([P, CPB, H], BF16, tag="osb")
            for j in range(CPB):
                for oc in range(H // CH):
                    op_ = ps.tile([P, CH], F32, tag="ps", name=f"o{ich0}_{j}_{oc}")
                    for h in range(HPC):
                        nc.tensor.matmul(op_, csb[:, h, j * P:(j + 1) * P],
                                         wo_sb[:, h, oc * CH:(oc + 1) * CH],
                                         start=(h == 0), stop=(h == HPC - 1))
                    nc.scalar.copy(o_sb[:, j, oc * CH:(oc + 1) * CH], op_)
            nc.scalar.dma_start(out=out_pr[ich0], in_=o_sb)

        for ich in range(NCH):
            b, li = ich // CPB, ich % CPB
            n0 = ich * CH

            xt_all = xt_pool.tile([P, KT, CH], BF16, tag="xta")
            nc.sync.dma_start(out=xt_all, in_=xTr[:, :, n0:n0 + CH])
            trig_raw = trig_pool.tile([P, 2, CH], BF16, tag="trig")
            nc.sync.dma_start(out=trig_raw, in_=trig_r[:, :, n0:n0 + CH])

            # ---- pass A: projections + x^2 stats over 16 d-tiles ----
            qp = [ps.tile([P, CH], F32, tag="ps", name=f"qp{ich}_{h}") for h in range(HPC)]
            kp = [ps.tile([P, CH], F32, tag="ps", name=f"kp{ich}_{h}") for h in range(HPC)]
            vp = [ps.tile([P, 2, OC], F32, tag="ps", name=f"vp{ich}_{g}") for g in range(2)]
            stats = ps.tile([P, CPB], F32, tag="ps", name=f"ss{ich}")
            xqs = {}

            def emit_stats(d):
                for j in range(CPB):
                    nc.tensor.matmul(stats[:, j:j + 1], xqs[d][:, j * P:(j + 1) * P],
                                     ones_sb, start=(d == 0), stop=(d == KT - 1),
                                     skip_group_check=True)

            for dt in range(KT):
                xt = xt_all[:, dt, :]
                xq = xq_pool.tile([P, CH], BF16, tag="xq")
                nc.vector.tensor_mul(xq, xt, xt)
                xqs[dt] = xq
                st, sp = (dt == 0), (dt == KT - 1)
                for h in range(HPC):
                    nc.tensor.matmul(qp[h], wq_sb[:, dt, h * P:(h + 1) * P], xt, start=st, stop=sp)
                    nc.tensor.matmul(kp[h], wk_sb[:, dt, h * P:(h + 1) * P], xt, start=st, stop=sp)
                for j in range(CPB):
                    nc.tensor.matmul(vp[j // 2][:, j % 2, :], xt[:, j * P:(j + 1) * P],
                                     wv_sb[:, dt, :], start=st, stop=sp,
                                     skip_group_check=True)
                if dt > 0:
                    emit_stats(dt - 1)
            emit_stats(KT - 1)

            # ---- RMS scale: s = exp(-0.5*ln(mean(x^2)+eps)) ----
            lnv = sc_pool.tile([P, CPB], F32, tag="ln")
            nc.scalar.activation(lnv, stats, LN, bias=eps_sb, scale=1.0 / H)
            s_col = sc_pool.tile([P, CPB], F32, tag="sc")
            nc.scalar.activation(s_col, lnv, EXP, scale=-0.5)
            s_rowT_p = ps.tile([CPB, P], F32, tag="ps", name=f"srt{ich}")
            nc.tensor.transpose(s_rowT_p, s_col, ident)
            s_rowT = sc_pool.tile([CPB, P], F32, tag="sr")
            nc.scalar.copy(s_rowT, s_rowT_p)
            s_bc = bc_pool.tile([P, CH], F32, tag="sbc")
            for j in range(CPB):
                nc.gpsimd.partition_broadcast(s_bc[:, j * P:(j + 1) * P], s_rowT[j:j + 1, :])
            cosS = trigs_pool.tile([P, CH], F32, tag="cosS")
            nc.vector.tensor_mul(cosS, trig_raw[:, 0, :], s_bc)
            sinS = trigs_pool.tile([P, CH], F32, tag="sinS")
            nc.vector.tensor_mul(sinS, trig_raw[:, 1, :], s_bc)

            # ---- V eviction: natural [token, dv] with per-partition RMS scale ----
            for j in range(CPB):
                nc.scalar.activation(v_chunks[ich][:, j, :], vp[j // 2][:, j % 2, :],
                                     COPY, scale=s_col[:, j:j + 1])

            # ---- partial o-proj of the previous chunk fills the RoPE bubble ----
            while pending:
                emit_oproj(*pending.pop(0))

            # ---- RoPE + scale eviction of q, k (psum [d,512] -> bf16 sbuf) ----
            q_sb = []
            for h in range(HPC):
                for (psum_t, dst) in ((qp[h], None), (kp[h], k_chunks[ich][:, h, :])):
                    t1 = rope_t.tile([P, CH], F32, tag="t1")
                    nc.vector.tensor_mul(t1, psum_t, cosS)
                    t2 = rope_t.tile([P, CH], F32, tag="t2")
                    nc.vector.tensor_mul(t2[0:HD, :], psum_t[HD:P, :], sinS[0:HD, :])
                    nc.vector.tensor_mul(t2[HD:P, :], psum_t[0:HD, :], sinS[HD:P, :])
                    if dst is None:
                        dst = q_pool.tile([P, CH], BF16, tag="q")
                        q_sb.append(dst)
                    nc.vector.tensor_add(dst, t1, t2)

            # ---- attention for this q-chunk, per head (pipelined, trimmed) ----
            nkt = CPB * (li + 1)
            ctx_sb = ctx_pool.tile([P, HPC, CH], BF16, tag="ctx")
            den = ps.tile([P, HPC * CPB], F32, tag="ps", name=f"dn{ich}")
            ctxs = []
            rec_cols = []
            for h in range(HPC):
                ctxp = ps.tile([P, CH], F32, tag="ps", name=f"cx{ich}_{h}")
                ctxs.append(ctxp)
                pend = []

                def flush_one(h=h, ctxp=ctxp):
                    kt0, qa0, ex0, ck0, j0 = pend.pop(0)
                    nc.tensor.matmul(ctxp[:, qa0:], v_chunks[ck0][:, j0, h * P:(h + 1) * P],
                                     ex0[:, qa0:], start=(kt0 == 0), stop=(kt0 == nkt - 1),
                                     skip_group_check=True)
                    kk0 = kt0 - CPB * li
                    for j2 in range(CPB):
                        if kk0 <= j2:
                            nc.tensor.matmul(den[:, h * CPB + j2:h * CPB + j2 + 1],
                                             ex0[:, j2 * P:(j2 + 1) * P], ones_sb,
                                             start=(kt0 == 0), stop=(kt0 == CPB * li + j2),
                                             skip_group_check=True)

                for kt in range(nkt):
                    ck = b * CPB + kt // CPB
                    j = kt % CPB
                    kk = kt - CPB * li
                    qa = kk * P if kk > 0 else 0
                    sp_ = ps.tile([P, CH], F32, tag="ps", name=f"s{ich}_{h}_{kt}")
                    nc.tensor.matmul(sp_[:, qa:], k_chunks[ck][:, h, j * P:(j + 1) * P],
                                     q_sb[h][:, qa:], start=True, stop=True,
                                     skip_group_check=True)
                    if kk >= 0:
                        nc.vector.tensor_add(sp_[:, kk * P:(kk + 1) * P],
                                             sp_[:, kk * P:(kk + 1) * P], mask_sb)
                    ex = ex_pool.tile([P, CH], BF16, tag="ex")
                    nc.scalar.activation(ex[:, qa:], sp_[:, qa:], EXP, scale=SCALE)
                    pend.append((kt, qa, ex, ck, j))
                    if len(pend) > 2:
                        flush_one()
                while pend:
                    flush_one()

                rec_col = sc_pool.tile([P, CPB], F32, tag="rc")
                nc.vector.reciprocal(rec_col, den[:, h * CPB:(h + 1) * CPB])
                rec_cols.append(rec_col)

            for h in range(HPC):
                recT_p = ps.tile([CPB, P], F32, tag="ps", name=f"rt{ich}_{h}")
                nc.tensor.transpose(recT_p, rec_cols[h], ident)
                recT = sc_pool.tile([CPB, P], F32, tag="rt")
                nc.scalar.copy(recT, recT_p)
                rbc = bc_pool.tile([P, CH], F32, tag="rbc")
                for j in range(CPB):
                    nc.gpsimd.partition_broadcast(rbc[:, j * P:(j + 1) * P], recT[j:j + 1, :])
                nc.vector.tensor_mul(ctx_sb[:, h, :], ctxs[h], rbc)

            pending.append((ich, ctx_sb))

        while pending:
            emit_oproj(*pending.pop(0))

    nc.compile()
    return nc


def prep_inputs(x, norm_w, wq, wk, wv, wo, position_ids):
    """Host-side sharding/layout prep. Returns per-core input maps."""
    import ml_dtypes
    bf16 = ml_dtypes.bfloat16
    x = np.asarray(x, dtype=np.float32)
    norm_w = np.asarray(norm_w, dtype=np.float32)
    wq = np.asarray(wq, dtype=np.float32)
    wk = np.asarray(wk, dtype=np.float32)
    wv = np.asarray(wv, dtype=np.float32)
    wo = np.asarray(wo, dtype=np.float32)
    pos = np.asarray(position_ids)

    xT = np.ascontiguousarray(x.reshape(NT, H).T).astype(bf16)

    # RoPE tables from position_ids, sign-folded sin
    inv_freq = 1.0 / (ROPE_BASE ** (np.arange(0, D, 2, dtype=np.float32) / D))
    t = pos.reshape(NT).astype(np.float32)
    freqs = np.einsum("n,f->nf", t, inv_freq)
    emb = np.concatenate([freqs, freqs], axis=1)          # [NT, D]
    cos = np.cos(emb).astype(np.float32)
    sin = np.sin(emb).astype(np.float32)
    sinF = sin.copy()
    sinF[:, :HD] *= -1.0
    trigT = np.stack([np.ascontiguousarray(cos.T),
                      np.ascontiguousarray(sinF.T)]).astype(bf16)   # [2, D, NT]

    # diagonal 128x128 triangle mask: mask[kp, qq] = 0 if qq >= kp else -1e4
    qq = np.arange(P)[None, :]
    kk = np.arange(P)[:, None]
    maskT = np.where(qq >= kk, 0.0, MASK_VAL).astype(bf16)

    onesT = np.ones((P, 1), dtype=bf16)

    wq_f = wq * norm_w[None, :]
    wk_f = wk * norm_w[None, :]
    wv_f = wv * norm_w[None, :]

    in_maps = []
    for c in range(NCORES):
        sl = slice(c * OC, (c + 1) * OC)
        in_maps.append({
            "xT": xT,
            "wqT": np.ascontiguousarray(wq_f[sl].T).astype(bf16),
            "wkT": np.ascontiguousarray(wk_f[sl].T).astype(bf16),
            "wvT": np.ascontiguousarray(wv_f[sl].T).astype(bf16),
            "woT": np.ascontiguousarray(wo[:, sl].T).astype(bf16),
            "trigT": trigT,
            "maskT": maskT,
            "onesT": onesT,
        })
    return in_maps


_NC_CACHE = None


def _get_module():
    global _NC_CACHE
    if _NC_CACHE is None:
        _NC_CACHE = build_module()
    return _NC_CACHE


def kernel(x, norm_w, wq, wk, wv, wo, position_ids):
    nc = _get_module()
    in_maps = prep_inputs(x, norm_w, wq, wk, wv, wo, position_ids)
    res = bass_utils.run_bass_kernel_spmd(nc, in_maps, core_ids=list(range(NCORES)))
    acc = np.zeros((NT, H), dtype=np.float32)
    for c in range(NCORES):
        acc += res.results[c]["out_p"].astype(np.float32)
    return acc.reshape(B, S, H)
